# revision 1
# baseline (speedup 1.0000x reference)
"""Trainium2 kernel for nn_Loss_HF_86079734546730.

Strategy (8 NeuronCores, SPMD, no collectives):
  - Shard the two [64,3,512,512] inputs spatially over H: core k gets raw
    rows [64k, 64k+64) => shard [64, 3, 64, 512] per tensor (48 MiB/core).
  - On device, per (tensor, channel, batch-pair) tile [128=(2b x 64h), 512w]:
      * DVE: horizontal Haar pass hs = x_even_w + x_odd_w, hd = x_even_w - x_odd_w
      * PE (fp32r, full fp32 precision at bf16 speed): one matmul per 128-col
        wb-chunk with a constant "vertical combo" matrix W. Because the
        stationary operand's free dim becomes the PSUM partition dim, this
        single matmul applies the vertical Haar combination AND transposes the
        result to [wb(spatial) x band-columns] layout -- exactly what the Gram
        stage needs for PE contraction over spatial.
      * ACT: copy PSUM -> SBUF band buffer with bf16 cast (single rounding).
  - PE: per (tensor, channel): Gram of the [spatial x (192 band cols + ones)]
    buffer accumulated in PSUM fp32 over all spatial chunks. The ones column
    makes row 192 of the Gram the per-band sums; the diagonal gives sum of
    squares. So means/stds/normalization/cosine-sim all reconstruct on the
    host from this single [193,193] matrix per (tensor, channel).
  - Host (float64): sum partial Grams over cores, rebuild per-(b,c,band)
    mean/std, expand the normalized-feature Gram P algebraically, cosine-sim,
    softmax, KL.
"""

import numpy as np
import ml_dtypes

B, C, H, W = 64, 3, 512, 512
NCORES = 8
HSH = H // NCORES          # 64 raw rows per core
NJ = B // 2                # 32 batch pairs
HB = HSH // 2              # 32 band rows per core
WB = W // 2                # 256 band cols
EPS_STD = 1e-5
EPS_COS = 1e-8
EPS_P = 1e-8

_CACHE = {}


def _make_w():
    """Constant matrices for the band-build matmuls.

    Rows: (b in 2) x (hr in 64)  [the partition layout of hs/hd tiles]
    wd cols (applied to hd): (b' in 2) x (band' in {lh,hh}) x (hb in 32)
        col = b'*64 + band'*32 + hb
        lh[hb] = ( hd[2hb] + hd[2hb+1]) / 2
        hh[hb] = (-hd[2hb] + hd[2hb+1]) / 2
    ws cols (applied to hs): (b' in 2) x (hb in 32), col = b'*32 + hb
        hl[hb] = ( hs[2hb] - hs[2hb+1]) / 2
    Two zero-padded [128, 256] fp32 operands packed as one [128, 512] param:
      w1 = [:, 0:256]   = [wd(128 cols) | zeros(128)]          applied to hd
      w2 = [:, 256:512] = [zeros(128) | ws(64) | zeros(64)]    applied to hs
    Both matmuls accumulate into the same full [128,256] PSUM region; the
    zero padding keeps each matmul's moving dim at 256 (fp32r full rate)
    without polluting the other's columns.
    """
    wmat = np.zeros((128, 512), np.float32)
    for b in range(2):
        for hb in range(32):
            r0 = b * 64 + 2 * hb
            r1 = r0 + 1
            c_lh = b * 64 + 0 * 32 + hb
            c_hh = b * 64 + 1 * 32 + hb
            wmat[r0, c_lh] = 0.5
            wmat[r1, c_lh] = 0.5
            wmat[r0, c_hh] = -0.5
            wmat[r1, c_hh] = 0.5
            c_hl = 256 + 128 + b * 32 + hb
            wmat[r0, c_hl] = 0.5
            wmat[r1, c_hl] = -0.5
    return wmat


def _band_col_map():
    """Map global band-buffer column g in [0,192) -> (batch, band).

    Tile j writes its 6 band columns (x32 hb each) contiguously at
    [192j, 192j+192) in the order produced by the W matmul + single
    contiguous ACT copy:
      local col block order: b0lh, b0hh, b1lh, b1hh, b0hl, b1hl
    Band buffer column index = bandcol*32 + hb with bandcol = 6j + local.
    """
    col_batch = np.zeros(192, np.int64)
    col_band = np.zeros(192, np.int64)
    for j in range(NJ):
        loc = [(0, 0), (0, 1), (1, 0), (1, 1), (0, 2), (1, 2)]
        for li, (bb, band) in enumerate(loc):
            g = 6 * j + li
            col_batch[g] = 2 * j + bb
            col_band[g] = band
    return col_batch, col_band


def _build_nc():
    import concourse.bass as bass
    import concourse.mybir as mybir
    import concourse.tile as tile
    from concourse import bacc

    f32 = mybir.dt.float32
    f32r = mybir.dt.float32r
    bf16 = mybir.dt.bfloat16

    nc = bacc.Bacc()
    za = nc.declare_dram_parameter("za", [B, C, HSH, W], f32, isOutput=False)
    zs = nc.declare_dram_parameter("zs", [B, C, HSH, W], f32, isOutput=False)
    wmat = nc.declare_dram_parameter("wmat", [128, 512], bf16, isOutput=False)
    gout = nc.declare_dram_parameter("G", [2, C, 193, 193], f32, isOutput=True)
    zz = [za, zs]

    NBCOL = 6176  # 192 band cols + 1 ones col, x32 hb each

    with tile.TileContext(nc) as tc:
        with (
            tc.tile_pool(name="wconst", bufs=1) as w_pool,
            tc.tile_pool(name="raw", bufs=4) as raw_pool,
            tc.tile_pool(name="hsd", bufs=4) as hsd_pool,
            tc.tile_pool(name="bands", bufs=2) as band_pool,
            tc.tile_pool(name="stage", bufs=4) as stage_pool,
            tc.tile_pool(name="pband", bufs=4, space="PSUM") as pb_pool,
            tc.tile_pool(name="pgram", bufs=1, space="PSUM") as pg_pool,
        ):
            w_t = w_pool.tile([128, 512], bf16, tag="wmat")
            nc.gpsimd.dma_start(w_t[:], wmat[:])
            w_r = w_t[:]

            for c in range(C):
                bufs = {}
                for t in range(2):
                    for wbc in range(2):
                        bb = band_pool.tile([128, NBCOL], bf16, tag=f"bb{t}{wbc}")
                        nc.gpsimd.memset(bb[:, 6144:6176], 1.0)
                        bufs[(t, wbc)] = bb

                for t in range(2):
                    for j in range(NJ):
                        raw = raw_pool.tile([128, W], f32, tag="raw")
                        nc.gpsimd.dma_start(raw[:], zz[t][2 * j : 2 * j + 2, c])
                        rw = raw[:].rearrange("p (w two) -> p w two", two=2)
                        hs = hsd_pool.tile([128, WB], bf16, tag="hs")
                        hd = hsd_pool.tile([128, WB], bf16, tag="hd")
                        nc.vector.tensor_add(hs[:], rw[:, :, 0], rw[:, :, 1])
                        nc.vector.tensor_sub(hd[:], rw[:, :, 0], rw[:, :, 1])
                        for wbc in range(2):
                            pband = pb_pool.tile([128, 256], f32, tag="pband")
                            # stationary = hd chunk [128, 128]: out partitions
                            # become wb (spatial); moving = combined W [128,256]
                            nc.tensor.matmul(
                                pband[:],
                                hd[:, 128 * wbc : 128 * (wbc + 1)],
                                w_r[:, 0:256],
                                start=True,
                                stop=False,
                            )
                            # hl needs hs as stationary; w2's zero padding
                            # protects the wd columns while accumulating.
                            nc.tensor.matmul(
                                pband[:],
                                hs[:, 128 * wbc : 128 * (wbc + 1)],
                                w_r[:, 256:512],
                                start=False,
                                stop=True,
                            )
                            # single contiguous copy: psum cols 0:192 are the
                            # 6 band blocks in host-known order; bf16 cast here
                            nc.scalar.activation(
                                bufs[(t, wbc)][:, 192 * j : 192 * j + 192],
                                pband[:, 0:192],
                                mybir.ActivationFunctionType.Copy,
                            )

                for t in range(2):
                    for chunk in range(2):
                        rows = 128 if chunk == 0 else 65
                        pg = pg_pool.tile([128, 193], f32, tag=f"pg{t}{chunk}")
                        for wbc in range(2):
                            bb3 = bufs[(t, wbc)][:].rearrange(
                                "p (col hb) -> p col hb", hb=32
                            )
                            for hb in range(32):
                                nc.tensor.matmul(
                                    pg[:rows, :],
                                    bb3[:, 128 * chunk : 128 * chunk + rows, hb],
                                    bb3[:, 0:193, hb],
                                    start=(wbc == 0 and hb == 0),
                                    stop=(wbc == 1 and hb == 31),
                                )
                        st = stage_pool.tile([128, 193], f32, tag="stage")
                        nc.vector.tensor_copy(st[:rows, :], pg[:rows, :])
                        nc.gpsimd.dma_start(
                            gout[t, c, 128 * chunk : 128 * chunk + rows, :],
                            st[:rows, :],
                        )
    if not nc.is_finalized():
        nc.finalize()
    return nc


def _get_nc():
    if "nc" not in _CACHE:
        _CACHE["nc"] = _build_nc()
    return _CACHE["nc"]


def _host_finish(g_parts):
    """g_parts: list of per-core G arrays [2,3,193,193] (fp32). Returns KL."""
    g = np.zeros((2, C, 193, 193), np.float64)
    for arr in g_parts:
        g += np.asarray(arr, np.float64)

    col_batch, col_band = _band_col_map()
    S = float(g[0, 0, 192, 192])

    P = np.zeros((2, B, B), np.float64)
    for t in range(2):
        for c in range(C):
            M = g[t, c, :192, :192]
            Tv = g[t, c, 192, :192]
            mu = Tv / S
            var = (np.diag(M) - Tv * Tv / S) / (S - 1.0)
            sig = np.sqrt(np.maximum(var, 0.0))
            alpha = 1.0 / (3.0 * (sig + EPS_STD))
            # centered gram of (t - mu) per column pair
            Mc = M - np.outer(mu, Tv) - np.outer(Tv, mu) + S * np.outer(mu, mu)
            Ms = (alpha[:, None] * Mc) * alpha[None, :]
            # group-sum columns into batches
            Bm = np.zeros((192, B), np.float64)
            Bm[np.arange(192), col_batch] = 1.0
            P[t] += Bm.T @ Ms @ Bm

    kls = []
    sims = []
    for t in range(2):
        r = np.sqrt(np.maximum(np.diag(P[t]), 0.0))
        rc = np.maximum(r, EPS_COS)
        sim = P[t] / np.outer(rc, rc)
        sims.append(sim)

    def logsoftmax_offdiag(sim):
        m = sim.copy()
        np.fill_diagonal(m, -np.inf)
        mx = m.max(axis=1, keepdims=True)
        e = np.exp(m - mx)
        ssum = e.sum(axis=1, keepdims=True)
        p = e / ssum
        return p

    p_ada = logsoftmax_offdiag(sims[0]) + EPS_P
    p_sou = logsoftmax_offdiag(sims[1]) + EPS_P
    kl = np.sum(p_sou * (np.log(p_sou) - np.log(p_ada))) / B
    return np.float32(kl)


def kernel(z_ada, z_sou):
    from concourse.bass_utils import run_bass_kernel_spmd

    z_ada = np.asarray(z_ada, np.float32)
    z_sou = np.asarray(z_sou, np.float32)
    wmat = _make_w().astype(ml_dtypes.bfloat16)

    in_maps = []
    for k in range(NCORES):
        sl = slice(HSH * k, HSH * (k + 1))
        in_maps.append(
            {
                "za": np.ascontiguousarray(z_ada[:, :, sl, :]),
                "zs": np.ascontiguousarray(z_sou[:, :, sl, :]),
                "wmat": wmat,
            }
        )

    nc = _get_nc()
    res = run_bass_kernel_spmd(nc, in_maps, list(range(NCORES)))
    g_parts = [res.results[k]["G"] for k in range(NCORES)]
    return _host_finish(g_parts)

